# revision 10
# baseline (speedup 1.0000x reference)
"""MultiHeadAttention Trainium2 kernel (8 NeuronCores, SPMD).

Reference computation (B=4, S=2048, DIM=1024, H=16, DEPTH=64):
    q = split_heads(Q @ Wq + bq); k = ...; v = ...
    logits = q k^T / sqrt(64) + mask * -1e9
    ctx = softmax(logits) @ v ; out = merge_heads(ctx) @ Wo + bo

Sharding: 8 cores = 4 batches x 2 query-halves. Each core computes the
full pipeline for (batch b, query rows qh*1024..+1024) over all 16 heads;
K/V projections for the batch are duplicated across the pair. No
cross-core communication.

Device dataflow (per core), all matmuls bf16 with fp32 PSUM accumulate:
  - Inputs stream per 128-row tile: DMA fp32 -> DVE cast bf16 -> one
    batched xbar DMA transpose ([128,1024] -> [128, 8, 128]), calls
    alternating between the two HWDGE queues (sync / scalar).
  - v = VT.T @ Wv + bv, consumed per key-tile as it lands, stored
    augmented per head as [ks, h, 65]: cols 0..63 = v_h * (1-mask[ks]),
    col 64 = (1-mask[ks]).  (mask + softmax denominator folded in)
  - qT = Wq.T @ QT [f, qs], kT = Wk.T @ KT [f, ks] -- projected per
    head-pair inside the attention loop so PE fills ACT-paced slack.
  - logitsT_h = kT_h.T @ qT_h [ks, qs] (K=64, two heads row-packed into
    one PSUM [128,1024] tile), exp on ACT straight from PSUM (bf16 out),
    ctxT_aug_h = v_aug_h.T @ attnT_h [65, qs]: row 64 = denominator.
  - Normalization deferred: unnormalized ctxT and denominators are
    copied out during attention; one batched reciprocal [16,1024], then
    per-head GPSIMD partition-broadcast + DVE multiply.
  - Y = ctxT.T @ Wo + bo [qs, f], natural layout, DMA out.
"""

from contextlib import ExitStack

import numpy as np

import concourse.bacc as bacc
import concourse.bass as bass
import concourse.tile as tile
from concourse import mybir
from concourse.bass_utils import run_bass_kernel_spmd

P = 128
B = 4
S = 2048
SQ = 1024  # query rows per core
DIM = 1024
H = 16
DEPTH = 64
NDT = DIM // P  # 8 feature/depth tiles
NKST = S // P  # 16 key tiles
NQST = SQ // P  # 8 query tiles
NHP = H // 2  # 8 head pairs

F32 = mybir.dt.float32
BF = mybir.dt.bfloat16
EXP = mybir.ActivationFunctionType.Exp


def _load_cast_w(nc, pool_stage, w_dram, w_bf):
    """DRAM fp32 [1024,1024] -> bf16 SBUF tile [128, 8, 1024] via casting
    SWDGE DMAs (one per 128-row block)."""
    for dt in range(NDT):
        nc.gpsimd.dma_start(
            out=w_bf[:, dt, :], in_=w_dram[dt * P : (dt + 1) * P, :]
        )


def _load_transpose_tile(nc, stage, stage_bf, x_dram, st_i, out3d):
    """One input s-tile: casting SWDGE DMA (fp32 -> bf16), then one batched
    xbar transpose (sync HWDGE only -- xbar mode stays pinned there)."""
    stb = stage_bf.tile([P, DIM], BF, tag="stage_bf", name=f"ldb{st_i}")
    nc.gpsimd.dma_start(out=stb[:], in_=x_dram[st_i * P : (st_i + 1) * P, :])
    nc.sync.dma_start_transpose(out=out3d, in_=stb[:])


def build_nc():
    nc = bacc.Bacc("TRN2", target_bir_lowering=False, debug=False, num_devices=8)

    qh_d = nc.dram_tensor("Qh", [SQ, DIM], F32, kind="ExternalInput").ap()
    kb_d = nc.dram_tensor("Kb", [S, DIM], F32, kind="ExternalInput").ap()
    vb_d = nc.dram_tensor("Vb", [S, DIM], F32, kind="ExternalInput").ap()
    mask_d = nc.dram_tensor("maskb", [S], F32, kind="ExternalInput").ap()
    wq_d = nc.dram_tensor("Wq", [DIM, DIM], F32, kind="ExternalInput").ap()
    wk_d = nc.dram_tensor("Wk", [DIM, DIM], F32, kind="ExternalInput").ap()
    wv_d = nc.dram_tensor("Wv", [DIM, DIM], F32, kind="ExternalInput").ap()
    wo_d = nc.dram_tensor("Wo", [DIM, DIM], F32, kind="ExternalInput").ap()
    bq_d = nc.dram_tensor("bq", [DIM], F32, kind="ExternalInput").ap()
    bk_d = nc.dram_tensor("bk", [DIM], F32, kind="ExternalInput").ap()
    bv_d = nc.dram_tensor("bv", [DIM], F32, kind="ExternalInput").ap()
    bo_d = nc.dram_tensor("bo", [DIM], F32, kind="ExternalInput").ap()
    y_d = nc.dram_tensor("Yh", [SQ, DIM], F32, kind="ExternalOutput").ap()

    with tile.TileContext(nc) as tc, ExitStack() as root:
        consts = root.enter_context(tc.tile_pool(name="consts", bufs=1))
        stage = None
        stage_bf = root.enter_context(tc.tile_pool(name="stage_bf", bufs=4))
        vaug_pool = root.enter_context(tc.tile_pool(name="vaug", bufs=1))
        ctxt_pool = root.enter_context(tc.tile_pool(name="ctxt", bufs=1))
        attn_pool = root.enter_context(tc.tile_pool(name="attn", bufs=3))
        norm_pool = root.enter_context(tc.tile_pool(name="norm", bufs=2))
        wo_pool = root.enter_context(tc.tile_pool(name="wo", bufs=1))

        # --- constants -------------------------------------------------
        ones_bf = consts.tile([1, P], BF)
        nc.vector.memset(ones_bf[:], 1.0)

        # mask -> m1 = 1 - mask, laid out [p, kst] (ks = kst*128 + p)
        m1_raw = consts.tile([P, NKST], F32)
        nc.sync.dma_start(
            out=m1_raw[:], in_=mask_d.rearrange("(t p) -> p t", p=P)
        )
        m1_pt = consts.tile([P, NKST], F32)
        nc.vector.tensor_scalar(
            out=m1_pt[:],
            in0=m1_raw[:],
            scalar1=-1.0,
            scalar2=1.0,
            op0=mybir.AluOpType.mult,
            op1=mybir.AluOpType.add,
        )
        m1_bf = consts.tile([P, NKST], BF)
        nc.vector.tensor_copy(out=m1_bf[:], in_=m1_pt[:])
        m1_rep = consts.tile([P, H, NKST], BF)  # replicated per head
        for h in range(H):
            nc.vector.tensor_copy(out=m1_rep[:, h, :], in_=m1_bf[:])

        # biases: bq/bk as per-partition scalars [p, ftile]
        bq_t = consts.tile([P, NDT], F32)
        nc.gpsimd.dma_start(out=bq_t[:], in_=bq_d.rearrange("(t p) -> p t", p=P))
        bk_t = consts.tile([P, NDT], F32)
        nc.gpsimd.dma_start(out=bk_t[:], in_=bk_d.rearrange("(t p) -> p t", p=P))
        # bv/bo as bf16 rows for K=1 PSUM-init matmuls (casting DMAs)
        bv_bf = consts.tile([1, DIM], BF)
        nc.gpsimd.dma_start(out=bv_bf[:], in_=bv_d[None, :])
        bo_bf = consts.tile([1, DIM], BF)
        nc.gpsimd.dma_start(out=bo_bf[:], in_=bo_d[None, :])

        wo_bf = wo_pool.tile([P, NDT, DIM], BF)
        _load_cast_w(nc, stage, wo_d, wo_bf)

        # v_aug: [p, kst, h, 65]
        v_aug_t = vaug_pool.tile([P, NKST, H, DEPTH + 1], BF)
        v_aug = [v_aug_t[:, t] for t in range(NKST)]
        # ctx^T (unnormalized until phase E): [dd(128), ddtile(8), qs(1024)]
        ctxT = ctxt_pool.tile([P, NDT, SQ], BF)
        # softmax denominators [h, qs] (gathered via sbuf-sbuf DMA)
        rs_all = consts.tile([H, SQ], F32)

        # --- phase A: V pipeline (load+transpose+project per tile) ----
        with ExitStack() as sv:
            wv_pool = sv.enter_context(tc.tile_pool(name="wv", bufs=1))
            vt_pool = sv.enter_context(tc.tile_pool(name="vt", bufs=3))
            psum_v = sv.enter_context(
                tc.tile_pool(name="psum_v", bufs=2, space="PSUM")
            )
            wv_bf = wv_pool.tile([P, NDT, DIM], BF)
            _load_cast_w(nc, stage, wv_d, wv_bf)

            for kst in range(NKST):
                vt_t = vt_pool.tile([P, NDT, P], BF, tag="vt", name=f"vt{kst}")
                _load_transpose_tile(nc, stage, stage_bf, vb_d, kst, vt_t[:])
                for half in range(2):  # dd columns [half*512, half*512+512)
                    ps = psum_v.tile([P, 512], F32, tag="psv", name=f"psv{kst}_{half}")
                    nc.tensor.matmul(
                        ps[:],
                        lhsT=ones_bf[:, :P],
                        rhs=bv_bf[:, half * 512 : half * 512 + 512],
                        start=True,
                        stop=False,
                    )
                    for dt in range(NDT):
                        nc.tensor.matmul(
                            ps[:],
                            lhsT=vt_t[:, dt, :],
                            rhs=wv_bf[:, dt, half * 512 : half * 512 + 512],
                            start=False,
                            stop=(dt == NDT - 1),
                        )
                    nc.vector.tensor_scalar(
                        out=v_aug[kst][:, half * 8 : half * 8 + 8, 0:DEPTH],
                        in0=ps.rearrange("p (h d) -> p h d", d=DEPTH),
                        scalar1=m1_pt[:, kst : kst + 1],
                        scalar2=None,
                        op0=mybir.AluOpType.mult,
                    )
                nc.vector.tensor_copy(
                    out=v_aug[kst][:, :, DEPTH : DEPTH + 1],
                    in_=m1_rep[:, :, kst : kst + 1],
                )

        # --- phases B-D: K/Q transposes, per-hp projections + attention
        with ExitStack() as sa:
            wq_pool = sa.enter_context(tc.tile_pool(name="wq", bufs=1))
            wk_pool = sa.enter_context(tc.tile_pool(name="wk", bufs=1))
            kt_pool = sa.enter_context(tc.tile_pool(name="kt", bufs=1))
            qt_pool = sa.enter_context(tc.tile_pool(name="qt", bufs=1))
            ktp_pool = sa.enter_context(tc.tile_pool(name="ktp", bufs=3))
            qtp_pool = sa.enter_context(tc.tile_pool(name="qtp", bufs=3))
            psum_p = sa.enter_context(
                tc.tile_pool(name="psum_p", bufs=2, space="PSUM")
            )
            psum_l = sa.enter_context(
                tc.tile_pool(name="psum_l", bufs=2, space="PSUM")
            )
            psum_c = sa.enter_context(
                tc.tile_pool(name="psum_c", bufs=1, space="PSUM")
            )

            wq_bf = wq_pool.tile([P, NDT, DIM], BF)
            _load_cast_w(nc, stage, wq_d, wq_bf)
            wk_bf = wk_pool.tile([P, NDT, DIM], BF)
            _load_cast_w(nc, stage, wk_d, wk_bf)
            kt_in = kt_pool.tile([P, NDT, S], BF)
            for st_i in range(NKST):
                _load_transpose_tile(
                    nc, stage, stage_bf, kb_d, st_i,
                    kt_in[:, :, st_i * P : (st_i + 1) * P],
                )
            qt_in = qt_pool.tile([P, NDT, SQ], BF)
            for st_i in range(NQST):
                _load_transpose_tile(
                    nc, stage, stage_bf, qh_d, st_i,
                    qt_in[:, :, st_i * P : (st_i + 1) * P],
                )

            for hp in range(NHP):
                ft = hp  # feature tile for this head pair
                kt_hp = ktp_pool.tile([P, S], BF, tag="kthp", name=f"kt{hp}")
                for ci in range(S // 512):
                    ps = psum_p.tile([P, 512], F32, tag="psp", name=f"psk{hp}_{ci}")
                    for dt in range(NDT):
                        nc.tensor.matmul(
                            ps[:],
                            lhsT=wk_bf[:, dt, ft * P : (ft + 1) * P],
                            rhs=kt_in[:, dt, ci * 512 : (ci + 1) * 512],
                            start=(dt == 0),
                            stop=(dt == NDT - 1),
                        )
                    nc.vector.tensor_scalar(
                        out=kt_hp[:, ci * 512 : (ci + 1) * 512],
                        in0=ps[:],
                        scalar1=bk_t[:, ft : ft + 1],
                        scalar2=None,
                        op0=mybir.AluOpType.add,
                    )
                qt_hp = qtp_pool.tile([P, SQ], BF, tag="qthp", name=f"qt{hp}")
                for ci in range(SQ // 512):
                    ps = psum_p.tile([P, 512], F32, tag="psp", name=f"psq{hp}_{ci}")
                    for dt in range(NDT):
                        nc.tensor.matmul(
                            ps[:],
                            lhsT=wq_bf[:, dt, ft * P : (ft + 1) * P],
                            rhs=qt_in[:, dt, ci * 512 : (ci + 1) * 512],
                            start=(dt == 0),
                            stop=(dt == NDT - 1),
                        )
                    nc.vector.tensor_scalar(
                        out=qt_hp[:, ci * 512 : (ci + 1) * 512],
                        in0=ps[:],
                        scalar1=bq_t[:, ft : ft + 1],
                        scalar2=None,
                        op0=mybir.AluOpType.add,
                    )

                # attention: heads h0 (kt rows 0:64), h1 (rows 64:128)
                h0, h1 = 2 * hp, 2 * hp + 1
                for qsi in range(2):
                    qs0 = qsi * 512
                    ctx0 = psum_c.tile([DEPTH + 1, 512], F32, tag="ctx0",
                                       name=f"c0_{hp}_{qsi}")
                    ctx1 = psum_c.tile([DEPTH + 1, 512], F32, tag="ctx1",
                                       name=f"c1_{hp}_{qsi}")
                    for kst in range(NKST):
                        pl = psum_l.tile([P, 1024], F32, tag="psl",
                                         name=f"pl{hp}_{qsi}_{kst}")
                        nc.tensor.matmul(
                            pl[:, 0:512],
                            lhsT=kt_hp[0:DEPTH, kst * P : (kst + 1) * P],
                            rhs=qt_hp[0:DEPTH, qs0 : qs0 + 512],
                            start=True,
                            stop=True,
                        )
                        nc.tensor.matmul(
                            pl[:, 512:1024],
                            lhsT=kt_hp[DEPTH:P, kst * P : (kst + 1) * P],
                            rhs=qt_hp[DEPTH:P, qs0 : qs0 + 512],
                            start=True,
                            stop=True,
                        )
                        at = attn_pool.tile([P, 1024], BF, tag="attnT",
                                            name=f"at{hp}_{qsi}_{kst}")
                        nc.scalar.activation(
                            out=at[:], in_=pl[:], func=EXP, scale=0.125
                        )
                        nc.tensor.matmul(
                            ctx0[:],
                            lhsT=v_aug[kst][:, h0, :],
                            rhs=at[:, 0:512],
                            start=(kst == 0),
                            stop=(kst == NKST - 1),
                        )
                        nc.tensor.matmul(
                            ctx1[:],
                            lhsT=v_aug[kst][:, h1, :],
                            rhs=at[:, 512:1024],
                            start=(kst == 0),
                            stop=(kst == NKST - 1),
                        )
                    # copy out unnormalized ctx + denominator row
                    for h, cps in ((h0, ctx0), (h1, ctx1)):
                        po = (h % 2) * DEPTH
                        nc.vector.tensor_copy(
                            out=ctxT[po : po + DEPTH, h // 2, qs0 : qs0 + 512],
                            in_=cps[0:DEPTH, :],
                        )
                        rrow = norm_pool.tile([1, 512], F32, tag="rrow",
                                              name=f"rr{hp}_{qsi}_{h}")
                        nc.vector.tensor_copy(
                            out=rrow[:], in_=cps[DEPTH : DEPTH + 1, :]
                        )
                        nc.sync.dma_start(
                            out=rs_all[h : h + 1, qs0 : qs0 + 512], in_=rrow[:]
                        )

        # --- phase E: batched normalization ---------------------------
        with ExitStack() as se:
            npool = se.enter_context(tc.tile_pool(name="npool", bufs=2))
            nc.vector.reciprocal(out=rs_all[:], in_=rs_all[:])
            rs_inv = rs_all
            for h in range(H):
                row0 = npool.tile([1, SQ], F32, tag="row0", name=f"r0_{h}")
                nc.gpsimd.dma_start(out=row0[:], in_=rs_inv[h : h + 1, :])
                rep = npool.tile([P, SQ], F32, tag="rep", name=f"rep{h}")
                nc.gpsimd.partition_broadcast(rep[:], row0[:])
                po = (h % 2) * DEPTH
                nc.vector.tensor_mul(
                    out=ctxT[po : po + DEPTH, h // 2, :],
                    in0=ctxT[po : po + DEPTH, h // 2, :],
                    in1=rep[po : po + DEPTH, :],
                )

        # --- phase F: out projection ----------------------------------
        with ExitStack() as so:
            psum_o = so.enter_context(
                tc.tile_pool(name="psum_o", bufs=2, space="PSUM")
            )
            y_pool = so.enter_context(tc.tile_pool(name="ysb", bufs=2))
            for qst in range(NQST):
                y_sb = y_pool.tile([P, DIM], F32, tag="ysb", name=f"y{qst}")
                for fh in range(2):
                    ps = psum_o.tile([P, 512], F32, tag="pso",
                                     name=f"po{qst}_{fh}")
                    nc.tensor.matmul(
                        ps[:],
                        lhsT=ones_bf[:, :P],
                        rhs=bo_bf[:, fh * 512 : fh * 512 + 512],
                        start=True,
                        stop=False,
                    )
                    for dt in range(NDT):
                        nc.tensor.matmul(
                            ps[:],
                            lhsT=ctxT[:, dt, qst * P : (qst + 1) * P],
                            rhs=wo_bf[:, dt, fh * 512 : fh * 512 + 512],
                            start=False,
                            stop=(dt == NDT - 1),
                        )
                    nc.vector.tensor_copy(
                        out=y_sb[:, fh * 512 : fh * 512 + 512], in_=ps[:]
                    )
                nc.sync.dma_start(
                    out=y_d[qst * P : (qst + 1) * P, :], in_=y_sb[:]
                )

    nc.compile()
    return nc


_NC_CACHE = None


def _get_nc():
    global _NC_CACHE
    if _NC_CACHE is None:
        _NC_CACHE = build_nc()
    return _NC_CACHE


def make_in_maps(Q, K, V, mask, Wq, bq, Wk, bk, Wv, bv, Wo, bo):
    f32 = lambda a: np.ascontiguousarray(np.asarray(a), dtype=np.float32)
    shared = {
        "Wq": f32(Wq), "Wk": f32(Wk), "Wv": f32(Wv), "Wo": f32(Wo),
        "bq": f32(bq), "bk": f32(bk), "bv": f32(bv), "bo": f32(bo),
    }
    Q, K, V, mask = f32(Q), f32(K), f32(V), f32(mask)
    in_maps = []
    for c in range(8):
        b, qh = c // 2, c % 2
        in_maps.append(
            {
                "Qh": np.ascontiguousarray(Q[b, qh * SQ : (qh + 1) * SQ, :]),
                "Kb": K[b],
                "Vb": V[b],
                "maskb": np.ascontiguousarray(mask[b, 0, 0, :]),
                **shared,
            }
        )
    return in_maps


def assemble(results):
    Y = np.empty((B, S, DIM), np.float32)
    for c in range(8):
        b, qh = c // 2, c % 2
        Y[b, qh * SQ : (qh + 1) * SQ, :] = results[c]["Yh"]
    return Y


def run_on_device(in_maps, trace=False, **kw):
    nc = _get_nc()
    return run_bass_kernel_spmd(nc, in_maps, list(range(8)), trace=trace, **kw)


def kernel(**inputs):
    in_maps = make_in_maps(**inputs)
    res = run_on_device(in_maps)
    return assemble(res.results)


# revision 11
# speedup vs baseline: 1.0723x; 1.0723x over previous
"""MultiHeadAttention Trainium2 kernel (8 NeuronCores, SPMD).

Reference computation (B=4, S=2048, DIM=1024, H=16, DEPTH=64):
    q = split_heads(Q @ Wq + bq); k = ...; v = ...
    logits = q k^T / sqrt(64) + mask * -1e9
    ctx = softmax(logits) @ v ; out = merge_heads(ctx) @ Wo + bo

Sharding: 8 cores = 4 batches x 2 query-halves. Each core computes the
full pipeline for (batch b, query rows qh*1024..+1024) over all 16 heads;
K/V projections for the batch are duplicated across the pair. No
cross-core communication.

Device dataflow (per core), all matmuls bf16 with fp32 PSUM accumulate:
  - Inputs stream per 128-row tile: DMA fp32 -> DVE cast bf16 -> one
    batched xbar DMA transpose ([128,1024] -> [128, 8, 128]), calls
    alternating between the two HWDGE queues (sync / scalar).
  - v = VT.T @ Wv + bv, consumed per key-tile as it lands, stored
    augmented per head as [ks, h, 65]: cols 0..63 = v_h * (1-mask[ks]),
    col 64 = (1-mask[ks]).  (mask + softmax denominator folded in)
  - qT = Wq.T @ QT [f, qs], kT = Wk.T @ KT [f, ks] -- projected per
    head-pair inside the attention loop so PE fills ACT-paced slack.
  - logitsT_h = kT_h.T @ qT_h [ks, qs] (K=64, two heads row-packed into
    one PSUM [128,1024] tile), exp on ACT straight from PSUM (bf16 out),
    ctxT_aug_h = v_aug_h.T @ attnT_h [65, qs]: row 64 = denominator.
  - Normalization deferred: unnormalized ctxT and denominators are
    copied out during attention; one batched reciprocal [16,1024], then
    per-head GPSIMD partition-broadcast + DVE multiply.
  - Y = ctxT.T @ Wo + bo [qs, f], natural layout, DMA out.
"""

from contextlib import ExitStack

import numpy as np

import concourse.bacc as bacc
import concourse.bass as bass
import concourse.tile as tile
from concourse import mybir
from concourse.bass_utils import run_bass_kernel_spmd

P = 128
B = 4
S = 2048
SQ = 1024  # query rows per core
DIM = 1024
H = 16
DEPTH = 64
NDT = DIM // P  # 8 feature/depth tiles
NKST = S // P  # 16 key tiles
NQST = SQ // P  # 8 query tiles
NHP = H // 2  # 8 head pairs

F32 = mybir.dt.float32
BF = mybir.dt.bfloat16
EXP = mybir.ActivationFunctionType.Exp


def _load_cast_w(nc, pool_stage, w_dram, w_bf):
    """DRAM fp32 [1024,1024] -> bf16 SBUF tile [128, 8, 1024] via casting
    SWDGE DMAs (one per 128-row block)."""
    for dt in range(NDT):
        nc.gpsimd.dma_start(
            out=w_bf[:, dt, :], in_=w_dram[dt * P : (dt + 1) * P, :]
        )


def _load_transpose_tile(nc, stage_bf, x_dram, st_i, out3d, tag):
    """One input s-tile: casting SWDGE DMA (fp32 -> bf16), then one batched
    xbar transpose (sync HWDGE only -- xbar mode stays pinned there)."""
    stb = stage_bf.tile([P, DIM], BF, tag=tag, name=f"{tag}{st_i}")
    nc.gpsimd.dma_start(out=stb[:], in_=x_dram[st_i * P : (st_i + 1) * P, :])
    nc.sync.dma_start_transpose(out=out3d, in_=stb[:])


def build_nc():
    nc = bacc.Bacc("TRN2", target_bir_lowering=False, debug=False, num_devices=8)

    qh_d = nc.dram_tensor("Qh", [SQ, DIM], F32, kind="ExternalInput").ap()
    kb_d = nc.dram_tensor("Kb", [S, DIM], F32, kind="ExternalInput").ap()
    vb_d = nc.dram_tensor("Vb", [S, DIM], F32, kind="ExternalInput").ap()
    mask_d = nc.dram_tensor("maskb", [S], F32, kind="ExternalInput").ap()
    wq_d = nc.dram_tensor("Wq", [DIM, DIM], F32, kind="ExternalInput").ap()
    wk_d = nc.dram_tensor("Wk", [DIM, DIM], F32, kind="ExternalInput").ap()
    wv_d = nc.dram_tensor("Wv", [DIM, DIM], F32, kind="ExternalInput").ap()
    wo_d = nc.dram_tensor("Wo", [DIM, DIM], F32, kind="ExternalInput").ap()
    bq_d = nc.dram_tensor("bq", [DIM], F32, kind="ExternalInput").ap()
    bk_d = nc.dram_tensor("bk", [DIM], F32, kind="ExternalInput").ap()
    bv_d = nc.dram_tensor("bv", [DIM], F32, kind="ExternalInput").ap()
    bo_d = nc.dram_tensor("bo", [DIM], F32, kind="ExternalInput").ap()
    y_d = nc.dram_tensor("Yh", [SQ, DIM], F32, kind="ExternalOutput").ap()

    with tile.TileContext(nc) as tc, ExitStack() as root:
        consts = root.enter_context(tc.tile_pool(name="consts", bufs=1))
        stage_bf = root.enter_context(tc.tile_pool(name="stage_bf", bufs=2))
        vaug_pool = root.enter_context(tc.tile_pool(name="vaug", bufs=1))
        ctxt_pool = root.enter_context(tc.tile_pool(name="ctxt", bufs=1))
        attn_pool = root.enter_context(tc.tile_pool(name="attn", bufs=3))
        norm_pool = root.enter_context(tc.tile_pool(name="norm", bufs=2))

        # --- constants -------------------------------------------------
        ones_bf = consts.tile([1, P], BF)
        nc.vector.memset(ones_bf[:], 1.0)

        # mask -> m1 = 1 - mask, laid out [p, kst] (ks = kst*128 + p)
        m1_raw = consts.tile([P, NKST], F32)
        nc.sync.dma_start(
            out=m1_raw[:], in_=mask_d.rearrange("(t p) -> p t", p=P)
        )
        m1_pt = consts.tile([P, NKST], F32)
        nc.vector.tensor_scalar(
            out=m1_pt[:],
            in0=m1_raw[:],
            scalar1=-1.0,
            scalar2=1.0,
            op0=mybir.AluOpType.mult,
            op1=mybir.AluOpType.add,
        )
        m1_bf = consts.tile([P, NKST], BF)
        nc.vector.tensor_copy(out=m1_bf[:], in_=m1_pt[:])
        m1_rep = consts.tile([P, H, NKST], BF)  # replicated per head
        for h in range(H):
            nc.vector.tensor_copy(out=m1_rep[:, h, :], in_=m1_bf[:])

        # biases: bq/bk as per-partition scalars [p, ftile]
        bq_t = consts.tile([P, NDT], F32)
        nc.gpsimd.dma_start(out=bq_t[:], in_=bq_d.rearrange("(t p) -> p t", p=P))
        bk_t = consts.tile([P, NDT], F32)
        nc.gpsimd.dma_start(out=bk_t[:], in_=bk_d.rearrange("(t p) -> p t", p=P))
        # bv/bo as bf16 rows for K=1 PSUM-init matmuls (casting DMAs)
        bv_bf = consts.tile([1, DIM], BF)
        nc.gpsimd.dma_start(out=bv_bf[:], in_=bv_d[None, :])
        bo_bf = consts.tile([1, DIM], BF)
        nc.gpsimd.dma_start(out=bo_bf[:], in_=bo_d[None, :])

        # v_aug: [p, kst, h, 65]
        v_aug_t = vaug_pool.tile([P, NKST, H, DEPTH + 1], BF)
        v_aug = [v_aug_t[:, t] for t in range(NKST)]
        # ctx^T (unnormalized until phase E): [dd(128), ddtile(8), qs(1024)]
        ctxT = ctxt_pool.tile([P, NDT, SQ], BF)
        # softmax denominators [h, qs] (gathered via sbuf-sbuf DMA)
        rs_all = consts.tile([H, SQ], F32)

        # --- phases: K/Q input streams, then V pipeline + projections
        # + attention, everything overlapped by the Tile scheduler -------
        with ExitStack() as sa:
            wq_pool = sa.enter_context(tc.tile_pool(name="wq", bufs=1))
            wk_pool = sa.enter_context(tc.tile_pool(name="wk", bufs=1))
            wv_pool = sa.enter_context(tc.tile_pool(name="wv", bufs=1))
            vt_pool = sa.enter_context(tc.tile_pool(name="vt", bufs=3))
            kt_pool = sa.enter_context(tc.tile_pool(name="kt", bufs=1))
            qt_pool = sa.enter_context(tc.tile_pool(name="qt", bufs=1))
            ktp_pool = sa.enter_context(tc.tile_pool(name="ktp", bufs=2))
            qtp_pool = sa.enter_context(tc.tile_pool(name="qtp", bufs=2))
            psum_p = sa.enter_context(
                tc.tile_pool(name="psum_p", bufs=2, space="PSUM")
            )
            psum_l = sa.enter_context(
                tc.tile_pool(name="psum_l", bufs=2, space="PSUM")
            )
            psum_c = sa.enter_context(
                tc.tile_pool(name="psum_c", bufs=1, space="PSUM")
            )

            # K and Q streams first: attention depends on them via kT/qT
            kt_in = kt_pool.tile([P, NDT, S], BF)
            for st_i in range(NKST):
                _load_transpose_tile(
                    nc, stage_bf, kb_d, st_i,
                    kt_in[:, :, st_i * P : (st_i + 1) * P], "ldbK",
                )
            qt_in = qt_pool.tile([P, NDT, SQ], BF)
            for st_i in range(NQST):
                _load_transpose_tile(
                    nc, stage_bf, qh_d, st_i,
                    qt_in[:, :, st_i * P : (st_i + 1) * P], "ldbQ",
                )
            wq_bf = wq_pool.tile([P, NDT, DIM], BF)
            _load_cast_w(nc, None, wq_d, wq_bf)
            wk_bf = wk_pool.tile([P, NDT, DIM], BF)
            _load_cast_w(nc, None, wk_d, wk_bf)
            wv_bf = wv_pool.tile([P, NDT, DIM], BF)
            _load_cast_w(nc, None, wv_d, wv_bf)

            # V pipeline: per key-tile load+transpose+project
            for kst in range(NKST):
                vt_t = vt_pool.tile([P, NDT, P], BF, tag="vt", name=f"vt{kst}")
                _load_transpose_tile(nc, stage_bf, vb_d, kst, vt_t[:], "ldbV")
                for half in range(2):  # dd columns [half*512, half*512+512)
                    ps = psum_p.tile([P, 512], F32, tag="psp",
                                     name=f"psv{kst}_{half}")
                    nc.tensor.matmul(
                        ps[:],
                        lhsT=ones_bf[:, :P],
                        rhs=bv_bf[:, half * 512 : half * 512 + 512],
                        start=True,
                        stop=False,
                    )
                    for dt in range(NDT):
                        nc.tensor.matmul(
                            ps[:],
                            lhsT=vt_t[:, dt, :],
                            rhs=wv_bf[:, dt, half * 512 : half * 512 + 512],
                            start=False,
                            stop=(dt == NDT - 1),
                        )
                    nc.vector.tensor_scalar(
                        out=v_aug[kst][:, half * 8 : half * 8 + 8, 0:DEPTH],
                        in0=ps.rearrange("p (h d) -> p h d", d=DEPTH),
                        scalar1=m1_pt[:, kst : kst + 1],
                        scalar2=None,
                        op0=mybir.AluOpType.mult,
                    )
                nc.vector.tensor_copy(
                    out=v_aug[kst][:, :, DEPTH : DEPTH + 1],
                    in_=m1_rep[:, :, kst : kst + 1],
                )

            for hp in range(NHP):
                ft = hp  # feature tile for this head pair
                kt_hp = ktp_pool.tile([P, S], BF, tag="kthp", name=f"kt{hp}")
                for ci in range(S // 512):
                    ps = psum_p.tile([P, 512], F32, tag="psp", name=f"psk{hp}_{ci}")
                    for dt in range(NDT):
                        nc.tensor.matmul(
                            ps[:],
                            lhsT=wk_bf[:, dt, ft * P : (ft + 1) * P],
                            rhs=kt_in[:, dt, ci * 512 : (ci + 1) * 512],
                            start=(dt == 0),
                            stop=(dt == NDT - 1),
                        )
                    nc.vector.tensor_scalar(
                        out=kt_hp[:, ci * 512 : (ci + 1) * 512],
                        in0=ps[:],
                        scalar1=bk_t[:, ft : ft + 1],
                        scalar2=None,
                        op0=mybir.AluOpType.add,
                    )
                qt_hp = qtp_pool.tile([P, SQ], BF, tag="qthp", name=f"qt{hp}")
                for ci in range(SQ // 512):
                    ps = psum_p.tile([P, 512], F32, tag="psp", name=f"psq{hp}_{ci}")
                    for dt in range(NDT):
                        nc.tensor.matmul(
                            ps[:],
                            lhsT=wq_bf[:, dt, ft * P : (ft + 1) * P],
                            rhs=qt_in[:, dt, ci * 512 : (ci + 1) * 512],
                            start=(dt == 0),
                            stop=(dt == NDT - 1),
                        )
                    nc.vector.tensor_scalar(
                        out=qt_hp[:, ci * 512 : (ci + 1) * 512],
                        in0=ps[:],
                        scalar1=bq_t[:, ft : ft + 1],
                        scalar2=None,
                        op0=mybir.AluOpType.add,
                    )

                # attention: heads h0 (kt rows 0:64), h1 (rows 64:128)
                h0, h1 = 2 * hp, 2 * hp + 1
                for qsi in range(2):
                    qs0 = qsi * 512
                    ctx0 = psum_c.tile([DEPTH + 1, 512], F32, tag="ctx0",
                                       name=f"c0_{hp}_{qsi}")
                    ctx1 = psum_c.tile([DEPTH + 1, 512], F32, tag="ctx1",
                                       name=f"c1_{hp}_{qsi}")
                    for kst in range(NKST):
                        pl = psum_l.tile([P, 1024], F32, tag="psl",
                                         name=f"pl{hp}_{qsi}_{kst}")
                        nc.tensor.matmul(
                            pl[:, 0:512],
                            lhsT=kt_hp[0:DEPTH, kst * P : (kst + 1) * P],
                            rhs=qt_hp[0:DEPTH, qs0 : qs0 + 512],
                            start=True,
                            stop=True,
                        )
                        nc.tensor.matmul(
                            pl[:, 512:1024],
                            lhsT=kt_hp[DEPTH:P, kst * P : (kst + 1) * P],
                            rhs=qt_hp[DEPTH:P, qs0 : qs0 + 512],
                            start=True,
                            stop=True,
                        )
                        at = attn_pool.tile([P, 1024], BF, tag="attnT",
                                            name=f"at{hp}_{qsi}_{kst}")
                        nc.scalar.activation(
                            out=at[:], in_=pl[:], func=EXP, scale=0.125
                        )
                        nc.tensor.matmul(
                            ctx0[:],
                            lhsT=v_aug[kst][:, h0, :],
                            rhs=at[:, 0:512],
                            start=(kst == 0),
                            stop=(kst == NKST - 1),
                        )
                        nc.tensor.matmul(
                            ctx1[:],
                            lhsT=v_aug[kst][:, h1, :],
                            rhs=at[:, 512:1024],
                            start=(kst == 0),
                            stop=(kst == NKST - 1),
                        )
                    # copy out unnormalized ctx + denominator row
                    for h, cps in ((h0, ctx0), (h1, ctx1)):
                        po = (h % 2) * DEPTH
                        nc.vector.tensor_copy(
                            out=ctxT[po : po + DEPTH, h // 2, qs0 : qs0 + 512],
                            in_=cps[0:DEPTH, :],
                        )
                        rrow = norm_pool.tile([1, 512], F32, tag="rrow",
                                              name=f"rr{hp}_{qsi}_{h}")
                        nc.vector.tensor_copy(
                            out=rrow[:], in_=cps[DEPTH : DEPTH + 1, :]
                        )
                        nc.sync.dma_start(
                            out=rs_all[h : h + 1, qs0 : qs0 + 512], in_=rrow[:]
                        )

        # --- tail: Wo load, batched normalization, out projection -----
        with ExitStack() as se:
            npool = se.enter_context(tc.tile_pool(name="npool", bufs=2))
            wo_pool = se.enter_context(tc.tile_pool(name="wo", bufs=1))
            wo_bf = wo_pool.tile([P, NDT, DIM], BF)
            _load_cast_w(nc, None, wo_d, wo_bf)
            nc.vector.reciprocal(out=rs_all[:], in_=rs_all[:])
            rs_inv = rs_all
            for h in range(H):
                row0 = npool.tile([1, SQ], F32, tag="row0", name=f"r0_{h}")
                nc.gpsimd.dma_start(out=row0[:], in_=rs_inv[h : h + 1, :])
                rep = npool.tile([P, SQ], F32, tag="rep", name=f"rep{h}")
                nc.gpsimd.partition_broadcast(rep[:], row0[:])
                po = (h % 2) * DEPTH
                nc.vector.tensor_mul(
                    out=ctxT[po : po + DEPTH, h // 2, :],
                    in0=ctxT[po : po + DEPTH, h // 2, :],
                    in1=rep[po : po + DEPTH, :],
                )

            # out projection
            psum_o = se.enter_context(
                tc.tile_pool(name="psum_o", bufs=2, space="PSUM")
            )
            y_pool = se.enter_context(tc.tile_pool(name="ysb", bufs=2))
            for qst in range(NQST):
                y_sb = y_pool.tile([P, DIM], F32, tag="ysb", name=f"y{qst}")
                for fh in range(2):
                    ps = psum_o.tile([P, 512], F32, tag="pso",
                                     name=f"po{qst}_{fh}")
                    nc.tensor.matmul(
                        ps[:],
                        lhsT=ones_bf[:, :P],
                        rhs=bo_bf[:, fh * 512 : fh * 512 + 512],
                        start=True,
                        stop=False,
                    )
                    for dt in range(NDT):
                        nc.tensor.matmul(
                            ps[:],
                            lhsT=ctxT[:, dt, qst * P : (qst + 1) * P],
                            rhs=wo_bf[:, dt, fh * 512 : fh * 512 + 512],
                            start=False,
                            stop=(dt == NDT - 1),
                        )
                    nc.vector.tensor_copy(
                        out=y_sb[:, fh * 512 : fh * 512 + 512], in_=ps[:]
                    )
                nc.sync.dma_start(
                    out=y_d[qst * P : (qst + 1) * P, :], in_=y_sb[:]
                )

    nc.compile()
    return nc


_NC_CACHE = None


def _get_nc():
    global _NC_CACHE
    if _NC_CACHE is None:
        _NC_CACHE = build_nc()
    return _NC_CACHE


def make_in_maps(Q, K, V, mask, Wq, bq, Wk, bk, Wv, bv, Wo, bo):
    f32 = lambda a: np.ascontiguousarray(np.asarray(a), dtype=np.float32)
    shared = {
        "Wq": f32(Wq), "Wk": f32(Wk), "Wv": f32(Wv), "Wo": f32(Wo),
        "bq": f32(bq), "bk": f32(bk), "bv": f32(bv), "bo": f32(bo),
    }
    Q, K, V, mask = f32(Q), f32(K), f32(V), f32(mask)
    in_maps = []
    for c in range(8):
        b, qh = c // 2, c % 2
        in_maps.append(
            {
                "Qh": np.ascontiguousarray(Q[b, qh * SQ : (qh + 1) * SQ, :]),
                "Kb": K[b],
                "Vb": V[b],
                "maskb": np.ascontiguousarray(mask[b, 0, 0, :]),
                **shared,
            }
        )
    return in_maps


def assemble(results):
    Y = np.empty((B, S, DIM), np.float32)
    for c in range(8):
        b, qh = c // 2, c % 2
        Y[b, qh * SQ : (qh + 1) * SQ, :] = results[c]["Yh"]
    return Y


def run_on_device(in_maps, trace=False, **kw):
    nc = _get_nc()
    return run_bass_kernel_spmd(nc, in_maps, list(range(8)), trace=trace, **kw)


def kernel(**inputs):
    in_maps = make_in_maps(**inputs)
    res = run_on_device(in_maps)
    return assemble(res.results)
